# revision 35
# baseline (speedup 1.0000x reference)
"""DotGAT 2-layer kernel, 8-core SPMD. ~1.92ms (baseline 3.15ms).

Late additions: pair-halves accumulate into the SAME PSUM columns via a
0-stride matmul output dim (verified on HW; kills the halves-merge at
finalize), layer-2 gather indices swapped in two halves with the first
issued mid-phase-1.

Structure per core (nodes degree-sorted, striped across cores; 49 stripes
of 128 own dst rows each; edges slotted dst-major, one 128-lane tile per
(stripe, degree-slot); int16 gather indices force pair-rows of 2 nodes):
- bf16 feature table for OWN nodes only (49 matmuls), single AllGather
  into table1 (identity layout, ~110us; only remaining serial stall).
- Edge phase: pair-gathers (1KB slots, 2 SWDGE queues), 2-tile-batched DVE
  prod/reduce/exp/mask/weight pipeline, single [128,512] PSUM-accumulate
  matmul per tile, halves summed at finalize.
- Layer-2 table rows computed inside layer-1 finalize (relu -> transpose
  -> 2 matmuls) and AllGathered in 7 chunks issued inline with the edge
  phase (fully hidden). table2 is chunk-major so each AG chunk's output is
  contiguous; layer 2 swaps in a second chunk-major index set in place.
- res / fd kept in SBUF bf16; fd stored x4 per stripe so batched DVE ops
  read contiguous spans. Pad rows get one live mask slot per head to keep
  softmax denominators finite (results discarded on host).
Bottleneck after all this: Vector engine at 99% (per-edge score+weight
elementwise work; the pair-row tax doubles it and is forced by the int16
dma_gather index range vs 50176 table rows).
"""
import json as _json
import numpy as np
import ml_dtypes

import concourse.bass as _cbass

if not getattr(_cbass.Bass, "_wait_split_patched", False):
    _orig_tjb = _cbass.Bass.to_json_bytes
    _ctr = [0]

    def _fix_block(insts):
        out = []
        for inst in insts:
            si = inst.get("sync_info")
            ow = (si or {}).get("on_wait") or []
            if si is not None and len(ow) > 1:
                for w in ow[:-1]:
                    _ctr[0] += 1
                    out.append({
                        "debug": inst.get("debug", 0),
                        "engine": inst["engine"],
                        "ins": [],
                        "name": f"WSPLIT-{_ctr[0]}-{inst['name']}",
                        "opcode": "EventSemaphore",
                        "outs": [],
                        "sync_info": {"on_update": [], "on_wait": [w]},
                    })
                si = dict(si)
                si["on_wait"] = [ow[-1]]
                inst = dict(inst)
                inst["sync_info"] = si
            out.append(inst)
        return out

    def _walk_fix(obj):
        if isinstance(obj, dict):
            if "instructions" in obj and isinstance(obj["instructions"], list):
                obj["instructions"] = _fix_block(obj["instructions"])
            for v in obj.values():
                _walk_fix(v)
        elif isinstance(obj, list):
            for v in obj:
                _walk_fix(v)

    def _patched_tjb(self, *a, **k):
        bir = _json.loads(_orig_tjb(self, *a, **k))
        _walk_fix(bir)
        return _json.dumps(bir).encode()

    _cbass.Bass.to_json_bytes = _patched_tjb
    _cbass.Bass._wait_split_patched = True

import concourse.bacc as bacc
import concourse.bass as bass
import concourse.mybir as mybir
from concourse.tile import TileContext
from concourse.tile_rust import add_dep_helper
from concourse._compat import get_trn_type
from concourse.bass_utils import run_bass_kernel_spmd
from concourse.library_config import mlp
from concourse.masks import make_identity

N, E, H, D = 50000, 500000, 4, 64
HD = H * D          # 256
P = 128
C = 8               # cores
STRIPE = P * C      # 1024
K_STRIPES = (N + STRIPE - 1) // STRIPE   # 49
NPAD = K_STRIPES * STRIPE                # 50176
OWN = K_STRIPES * P                      # 6272 rows per core
NIDX = 1024                              # slots per dma_gather call (8 tiles)
NCHUNK = 7                               # AG2 chunks (49 = 7*7 stripes)
CHS = K_STRIPES // NCHUNK                # stripes per chunk
F32 = mybir.dt.float32
BF16 = mybir.dt.bfloat16
I16 = mybir.dt.int16
BF = ml_dtypes.bfloat16


def _apx(base_ap, col0, dims):
    """AP at free-column col0 of a [128, F] tile with custom free dims."""
    return bass.AP(base_ap.tensor, base_ap.offset + col0,
                   [base_ap.ap[0]] + dims)


# ---------------------------------------------------------------- host prep
def _prepare(src, dst):
    deg = np.bincount(dst, minlength=N)
    perm = np.argsort(-deg, kind="stable").astype(np.int64)     # rank -> node
    ranks = np.arange(NPAD)
    k_of = ranks // STRIPE
    c_of = (ranks % STRIPE) // P
    p_of = ranks % P
    agrow_of_rank = c_of * OWN + k_of * P + p_of
    node_at_ag = np.zeros(NPAD, dtype=np.int64)
    node_at_ag[agrow_of_rank] = perm[np.minimum(ranks, N - 1)]
    ag_of_node = np.zeros(N, dtype=np.int64)
    ag_of_node[perm[np.arange(N)]] = agrow_of_rank[np.arange(N)]

    deg_r = np.zeros(NPAD, dtype=np.int64)
    deg_r[:N] = deg[perm]
    T = deg_r.reshape(K_STRIPES, STRIPE).max(axis=1)
    NT = int(T.sum())
    NCALLS = -(-NT // 8)
    NTP = NCALLS * 8
    Tp = T.copy()
    Tp[-1] += NTP - NT
    tile0 = np.zeros(K_STRIPES + 1, dtype=np.int64)
    np.cumsum(Tp, out=tile0[1:])

    order = np.argsort(dst, kind="stable")
    src_s = src[order]
    starts = np.zeros(N + 1, dtype=np.int64)
    np.cumsum(deg, out=starts[1:])

    # chunk-major table layout: ag row r=(c,k,p) -> chunk g=k//CHS block
    rr = np.arange(NPAD)
    c_r, rem = rr // OWN, rr % OWN
    k_r, p_r = rem // P, rem % P
    rowmap = ((k_r // CHS) * (C * CHS * P) + c_r * (CHS * P)
              + (k_r % CHS) * P + p_r)
    ag2 = rowmap[ag_of_node]                      # node -> chunk-major row

    gidx1 = np.zeros((C, P, NCALLS * 64), dtype=np.int16)
    gidx2 = np.zeros((C, P, NCALLS * 64), dtype=np.int16)
    mask8 = np.zeros((C, P, 8 * NTP), dtype=np.float32)
    for c in range(C):
        flat1 = np.zeros(NTP * P, dtype=np.int16)   # slot i = t*128+p
        flat2 = np.zeros(NTP * P, dtype=np.int16)
        for k in range(K_STRIPES):
            Tk = int(Tp[k])
            base_t = int(tile0[k])
            r0 = k * STRIPE + c * P
            colbase = 8 * base_t
            for p in range(P):
                r = r0 + p
                if r >= N:
                    # pad row: keep one live mask slot per head so the
                    # denominator stays finite (result discarded on host)
                    for h in range(H):
                        mask8[c, p, colbase + h * 2 * Tk] = 1.0
                    continue
                node = perm[r]
                d0 = starts[node]
                g = min(int(deg[node]), Tk)
                srcs = src_s[d0:d0 + g]
                a1 = ag_of_node[srcs]     # identity layout (table1)
                a2 = ag2[srcs]            # chunk-major layout (table2)
                for t in range(g):
                    flat1[(base_t + t) * P + p] = a1[t] >> 1
                    flat2[(base_t + t) * P + p] = a2[t] >> 1
                    b = int(a1[t] & 1)
                    for h in range(H):
                        mask8[c, p, colbase + h * 2 * Tk + 2 * t + b] = 1.0
        for flat, gx in ((flat1, gidx1), (flat2, gidx2)):
            w = flat.reshape(NCALLS, 64, 16)
            for call in range(NCALLS):
                gx[c, :, call * 64:(call + 1) * 64] = np.tile(w[call].T, (8, 1))
    return dict(perm=perm, node_at_ag=node_at_ag, ag_of_node=ag_of_node,
                T=T, Tp=Tp, tile0=tile0, NT=NT, NCALLS=NCALLS, NTP=NTP,
                gidx1=gidx1, gidx2=gidx2, mask8=mask8.astype(BF))


# ------------------------------------------------------------- device build
def _build(meta):
    NCALLS, NTP = meta["NCALLS"], meta["NTP"]
    Tp, tile0 = meta["Tp"], meta["tile0"]
    EXCOLS = 8 * int(Tp.max())
    tile_stripe = np.zeros(NTP, dtype=np.int64)
    for k in range(K_STRIPES):
        tile_stripe[tile0[k]:tile0[k + 1]] = k

    nc = bacc.Bacc(get_trn_type() or "TRN2", num_swdge_queues=2)
    xoT_d = nc.dram_tensor("xoT", [P, OWN], BF16, kind="ExternalInput")
    W1 = nc.dram_tensor("W1", [P, HD], BF16, kind="ExternalInput")
    Wres1 = nc.dram_tensor("Wres1", [P, HD], BF16, kind="ExternalInput")
    W2 = nc.dram_tensor("W2", [HD, HD], BF16, kind="ExternalInput")
    gidx_d = nc.dram_tensor("gidx", [P, NCALLS * 64], I16, kind="ExternalInput")
    gidx2_d = nc.dram_tensor("gidx2", [P, NCALLS * 64], I16, kind="ExternalInput")
    mask8_d = nc.dram_tensor("mask8", [P, 8 * NTP], BF16, kind="ExternalInput")
    out_own = nc.dram_tensor("out_own", [OWN, HD], F32, kind="ExternalOutput")

    t1own = nc.dram_tensor("t1own", [OWN, HD], BF16)
    t2own = nc.dram_tensor("t2own", [OWN, HD], BF16)
    table1 = nc.dram_tensor("table1", [NPAD, HD], BF16, addr_space="Shared")
    table2 = nc.dram_tensor("table2", [NPAD, HD], BF16, addr_space="Shared")

    with TileContext(nc) as tc:
        with tc.tile_pool(name="const", bufs=1) as cpool, \
             tc.tile_pool(name="fd", bufs=1) as fdpool, \
             tc.tile_pool(name="tstage", bufs=4) as tstage, \
             tc.tile_pool(name="edge", bufs=2) as epool, \
             tc.tile_pool(name="small", bufs=8) as spool, \
             tc.tile_pool(name="exb", bufs=2) as expool, \
             tc.tile_pool(name="pstab", bufs=2, space="PSUM") as pstab, \
             tc.tile_pool(name="psagg", bufs=2, space="PSUM") as psagg, \
             tc.tile_pool(name="pstr", bufs=1, space="PSUM") as pstr:

            lib = nc.gpsimd.load_library(mlp)
            nidx_reg = nc.gpsimd.to_reg(NIDX)

            w1r = cpool.tile([P, 2 * HD], BF16)
            nc.sync.dma_start(out=w1r[:, 0:HD], in_=W1[:])
            nc.sync.dma_start(out=w1r[:, HD:], in_=Wres1[:])
            w2a = cpool.tile([P, HD], BF16)
            nc.sync.dma_start(out=w2a[:], in_=W2[0:P, :])
            w2b = cpool.tile([P, HD], BF16)
            nc.sync.dma_start(out=w2b[:], in_=W2[P:HD, :])
            identb = cpool.tile([P, P], BF16)
            make_identity(nc, identb[:])
            gixt = cpool.tile([P, NCALLS * 64], I16)
            nc.sync.dma_start(out=gixt[:], in_=gidx_d[:])
            m8 = cpool.tile([P, 8 * NTP], BF16)
            nc.sync.dma_start(out=m8[:], in_=mask8_d[:])
            xoT = cpool.tile([P, OWN], BF16)
            nc.sync.dma_start(out=xoT[:], in_=xoT_d[:])

            # fd features x4 per stripe: [fd,fd,fd,fd] -> 1024 cols per stripe
            fdbuf = fdpool.tile([P, K_STRIPES * 4 * HD], BF16)
            resbuf = fdpool.tile([P, K_STRIPES * HD], BF16)

            def ag_chunk(table_own, table_full, gch, writes, tag):
                r0 = gch * CHS * P
                r1 = (gch + 1) * CHS * P
                own_ap = table_own[r0:r1, :]
                full_ap = bass.AP(table_full[:].tensor, r0 * C * HD,
                                  [[1, (r1 - r0) * C * HD]])
                cc = nc.gpsimd.collective_compute(
                    "AllGather", mybir.AluOpType.bypass,
                    replica_groups=[list(range(C))],
                    ins=[own_ap], outs=[full_ap])
                for wi in writes:
                    add_dep_helper(cc.ins, wi, True, tag)
                return cc.ins

            # ---- own-shard table1 + residual build (49 fused matmuls) ----
            t1_writes = []
            for k in range(K_STRIPES):
                ps = pstab.tile([P, 2 * HD], F32, tag="tab")
                nc.tensor.matmul(out=ps[:], lhsT=xoT[:, k * P:(k + 1) * P],
                                 rhs=w1r[:], start=True, stop=True)
                st = tstage.tile([P, HD], BF16, tag="tst")
                nc.scalar.copy(out=st[:], in_=ps[:, 0:HD])
                wr = nc.scalar.dma_start(out=t1own[k * P:(k + 1) * P, :], in_=st[:])
                t1_writes.append(wr.ins)
                nc.vector.tensor_copy(
                    out=_apx(fdbuf[:], k * 4 * HD, [[HD, 4], [1, HD]]),
                    in_=_apx(ps[:], 0, [[0, 4], [1, HD]]))
                nc.vector.tensor_copy(
                    out=resbuf[:, k * HD:(k + 1) * HD], in_=ps[:, HD:])
            cc1 = nc.gpsimd.collective_compute(
                "AllGather", mybir.AluOpType.bypass,
                replica_groups=[list(range(C))],
                ins=[t1own[:]], outs=[table1[:]])
            for wi in t1_writes:
                add_dep_helper(cc1.ins, wi, True, "t1->cc1")
            cc1_ins = [cc1.ins]

            chunk_writes = [[] for _ in range(NCHUNK)]

            def edge_phase(layer, table, barrier_insts, on_chunk=None):
                tablev = table[:].rearrange("(a b) c -> a (b c)", b=2)
                cur = {}

                def finalize(k):
                    Tk = int(Tp[k])
                    exm = cur["exm"]
                    agg = cur["agg"]
                    den = spool.tile([P, H], F32, tag="den")
                    nc.vector.tensor_reduce(
                        out=den[:],
                        in_=_apx(exm[:], 0, [[2 * Tk, H], [1, 2 * Tk]]),
                        axis=mybir.AxisListType.X, op=mybir.AluOpType.add)
                    rec = spool.tile([P, H], F32, tag="rec")
                    nc.vector.reciprocal(out=rec[:], in_=den[:])
                    st = tstage.tile([P, HD], F32, tag="fin")
                    nc.vector.tensor_tensor(
                        out=st[:].rearrange("p (h d) -> p h d", h=H),
                        in0=agg[:].rearrange("p (h d) -> p h d", h=H),
                        in1=_apx(rec[:], 0, [[1, H], [0, D]]),
                        op=mybir.AluOpType.mult)
                    if layer == 1:
                        nc.vector.tensor_add(out=st[:], in0=st[:],
                                             in1=resbuf[:, k * HD:(k + 1) * HD])
                        stb = tstage.tile([P, HD], BF16, tag="stb")
                        nc.scalar.activation(
                            out=stb[:], in_=st[:],
                            func=mybir.ActivationFunctionType.Relu)
                        lts = []
                        for q in range(2):
                            tp = pstr.tile([P, P], BF16, tag="tr")
                            nc.tensor.transpose(out=tp[:],
                                                in_=stb[:, q * P:(q + 1) * P],
                                                identity=identb[:])
                            lt = tstage.tile([P, P], BF16, tag="trs")
                            nc.scalar.copy(out=lt[:], in_=tp[:])
                            lts.append(lt)
                        ps2 = pstab.tile([P, HD], F32, tag="t2")
                        nc.tensor.matmul(out=ps2[:], lhsT=lts[0][:], rhs=w2a[:],
                                         start=True, stop=False)
                        nc.tensor.matmul(out=ps2[:], lhsT=lts[1][:], rhs=w2b[:],
                                         start=False, stop=True)
                        st2 = tstage.tile([P, HD], BF16, tag="tst")
                        nc.scalar.copy(out=st2[:], in_=ps2[:])
                        wr = nc.scalar.dma_start(
                            out=t2own[k * P:(k + 1) * P, :], in_=st2[:])
                        chunk_writes[min(k // CHS, NCHUNK - 1)].append(wr.ins)
                        nc.scalar.copy(
                            out=_apx(fdbuf[:], k * 4 * HD, [[HD, 4], [1, HD]]),
                            in_=_apx(st2[:], 0, [[0, 4], [1, HD]]))
                        if on_chunk is not None and k % CHS == CHS - 1:
                            on_chunk(k // CHS)
                    else:
                        nc.sync.dma_start(out=out_own[k * P:(k + 1) * P, :],
                                          in_=st[:])

                for call in range(NCALLS):
                    fs2 = epool.tile([P, 8, 2 * HD], BF16, tag="fs2", bufs=3)
                    g = nc.gpsimd.dma_gather(
                        fs2[:], tablev, gixt[:, call * 64:(call + 1) * 64],
                        NIDX, nidx_reg, 2 * HD, transpose=False,
                        single_packet=False, queue_num=call % 2)
                    add_dep_helper(g.ins, lib.ins, True, "lib->gather")
                    for bi in barrier_insts:
                        add_dep_helper(g.ins, bi, True, "table->gather")
                    tl = 0
                    while tl < 8:
                        t = call * 8 + tl
                        k = int(tile_stripe[t])
                        t_local = t - int(tile0[k])
                        Tk = int(Tp[k])
                        # batch 2 tiles when both sit in this call + stripe
                        nt = 2 if (tl + 1 < 8 and t_local + 1 < Tk) else 1
                        W = nt * 2 * HD
                        if t_local == 0:
                            agg_t = psagg.tile([P, HD], F32, tag="agg")
                            exm_t = expool.tile([P, EXCOLS], BF16, tag="exm")
                            cur["agg"] = agg_t
                            cur["exm"] = exm_t
                        agg = cur["agg"]
                        exm = cur["exm"]
                        prod = epool.tile([P, 4 * HD], BF16, tag="prod")
                        nc.vector.tensor_tensor(
                            out=_apx(prod[:], 0, [[1, W]]),
                            in0=_apx(fs2[:], tl * 2 * HD, [[1, W]]),
                            in1=_apx(fdbuf[:], k * 4 * HD, [[1, W]]),
                            op=mybir.AluOpType.mult)
                        sc = spool.tile([P, 4 * H], F32, tag="sc")
                        nc.vector.tensor_reduce(
                            out=_apx(sc[:], 0, [[1, nt * 2 * H]]),
                            in_=_apx(prod[:], 0, [[D, nt * 2 * H], [1, D]]),
                            axis=mybir.AxisListType.X, op=mybir.AluOpType.add)
                        ex = spool.tile([P, 4 * H], BF16, tag="ex")
                        nc.scalar.activation(
                            out=_apx(ex[:], 0, [[1, nt * 2 * H]]),
                            in_=_apx(sc[:], 0, [[1, nt * 2 * H]]),
                            func=mybir.ActivationFunctionType.Exp,
                            scale=0.125)
                        exm_ap = _apx(exm[:], 2 * t_local,
                                      [[2 * Tk, H], [1, 2 * nt]])
                        ex_ap = _apx(ex[:], 0, [[1, H], [H, 2 * nt]])
                        m8_ap = _apx(m8[:], 8 * int(tile0[k]) + 2 * t_local,
                                     [[2 * Tk, H], [1, 2 * nt]])
                        nc.vector.tensor_tensor(
                            out=exm_ap, in0=ex_ap, in1=m8_ap,
                            op=mybir.AluOpType.mult)
                        ws2 = epool.tile([P, 4 * HD], BF16, tag="ws2")
                        nc.vector.tensor_tensor(
                            out=_apx(ws2[:], 0, [[HD, 2 * nt], [D, H], [1, D]]),
                            in0=_apx(fs2[:], tl * 2 * HD,
                                     [[HD, 2 * nt], [D, H], [1, D]]),
                            in1=_apx(exm[:], 2 * t_local,
                                     [[1, 2 * nt], [2 * Tk, H], [0, D]]),
                            op=mybir.AluOpType.mult)
                        for j in range(nt):
                            if t_local + j == 0:
                                # first tile: unambiguous start semantics on
                                # the two pair-halves (reset, then accumulate)
                                nc.tensor.matmul(
                                    out=agg[:], lhsT=identb[:],
                                    rhs=ws2[:, j * 2 * HD:j * 2 * HD + HD],
                                    start=True, stop=False)
                                nc.tensor.matmul(
                                    out=agg[:], lhsT=identb[:],
                                    rhs=ws2[:, j * 2 * HD + HD:(j + 1) * 2 * HD],
                                    start=False, stop=(t_local + j == Tk - 1))
                            else:
                                # both pair-halves accumulate into the same
                                # PSUM columns via a 0-stride output dim
                                nc.tensor.matmul(
                                    out=_apx(agg[:], 0, [[0, 2], [1, HD]]),
                                    lhsT=identb[:],
                                    rhs=ws2[:, j * 2 * HD:(j + 1) * 2 * HD],
                                    start=False,
                                    stop=(t_local + j == Tk - 1))
                        if t_local + nt == Tk:
                            finalize(k)
                        tl += nt
                    if layer == 1 and call == NCALLS // 2 - 1:
                        # early swap of the first half of the layer-2 gather
                        # indices (WAR on calls 0..31 handled by Tile)
                        nc.sync.dma_start(
                            out=gixt[:, :(NCALLS // 2) * 64],
                            in_=gidx2_d[:, :(NCALLS // 2) * 64])

            # ---------------- layer 1 (chunked AG2 issued inline) --------
            cc2_ins = []

            def issue_cc2(gch):
                cc2_ins.append(ag_chunk(t2own, table2, gch,
                                        chunk_writes[gch], "t2->cc2"))

            edge_phase(1, table1, tuple(cc1_ins), on_chunk=issue_cc2)
            assert len(cc2_ins) == NCHUNK

            # second half of the chunk-major gather indices for layer 2
            nc.sync.dma_start(out=gixt[:, (NCALLS // 2) * 64:],
                              in_=gidx2_d[:, (NCALLS // 2) * 64:])

            # ---------------- layer 2 ----------------
            edge_phase(2, table2, tuple(cc2_ins))

    nc.compile()
    return nc


_CACHE = {}


def _get_built(src, dst):
    key = (int(src[:16].sum()), int(dst[:16].sum()), int(src.sum()), int(dst.sum()))
    if key not in _CACHE:
        meta = _prepare(np.asarray(src, dtype=np.int64),
                        np.asarray(dst, dtype=np.int64))
        nc = _build(meta)
        _CACHE[key] = (meta, nc)
    return _CACHE[key]


def _run(x, src, dst, W1, Wres1, W2, trace=False):
    meta, nc = _get_built(np.asarray(src), np.asarray(dst))
    node_at_ag = meta["node_at_ag"]
    x = np.asarray(x, dtype=np.float32)
    x_ag = x[node_at_ag].astype(BF)                # [NPAD, 128] bf16
    W1b = np.ascontiguousarray(np.asarray(W1, dtype=np.float32).astype(BF))
    Wres1b = np.ascontiguousarray(np.asarray(Wres1, dtype=np.float32).astype(BF))
    W2b = np.ascontiguousarray(np.asarray(W2, dtype=np.float32).astype(BF))
    in_maps = []
    for c in range(C):
        xoT = np.ascontiguousarray(x_ag[c * OWN:(c + 1) * OWN].T)
        in_maps.append({
            "xoT": xoT,
            "W1": W1b,
            "Wres1": Wres1b,
            "W2": W2b,
            "gidx": meta["gidx1"][c],
            "gidx2": meta["gidx2"][c],
            "mask8": meta["mask8"][c],
        })
    res = run_bass_kernel_spmd(nc, in_maps, core_ids=list(range(C)), trace=trace)
    out = np.zeros((N, HD), dtype=np.float32)
    for c in range(C):
        rows = res.results[c]["out_own"]       # [OWN, 256], ag rows of core c
        nodes = node_at_ag[c * OWN:(c + 1) * OWN]
        loc = np.arange(OWN)
        rr = (loc // P) * STRIPE + c * P + (loc % P)   # global rank
        valid = rr < N
        out[nodes[valid]] = rows[valid]
    return out, res.exec_time_ns


def kernel(x, src, dst, W1, Wres1, W2):
    out, _ = _run(x, src, dst, W1, Wres1, W2, trace=False)
    return out


def kernel_traced(x, src, dst, W1, Wres1, W2):
    return _run(x, src, dst, W1, Wres1, W2, trace=True)
